# revision 47
# baseline (speedup 1.0000x reference)
"""Trainium2 Bass/Tile kernel for a dense transformer block.

Math (per batch element b, T=16 tokens, C=512, H=8 heads, D=64):
    h  = LN(x; ln1_g, ln1_b)
    q,k,v = per-head projections of h
    att = causal-softmax(q k^T / sqrt(D)); o = att v (heads concatenated)
    y  = o @ w_proj + b_proj + x
    f  = relu(LN(y; ln2_g, ln2_b) @ w1 + b1) @ w2 + b2
    out = f + y

Distribution: pure data parallel over the batch dim (4096) across 8
NeuronCores; weights replicated; no collectives.

Device layout strategy (per core, 512 batch elems = 8192 tokens):
  * tokens-on-partitions for x, LN, v, y, out (128 tokens/tile, 64 tiles)
  * feature-major (C-on-partitions) for matmul moving operands, obtained
    with PE transposes; q^T/k^T produced feature-major directly by
    weight-stationary matmuls so per-head [64 x 128] slices feed the
    attention matmuls with no extra copies.
  * attention per 128-token tile (8 whole batch elems): per-head
    [128, 128] logits via K=64 row-tiled matmuls (tile_position packs two
    heads into disjoint row-halves of the PE array so they run
    concurrently); exp on ScalarE; block-diag causal mask + row sums on
    VectorE; att^T via the DVE 32x32 stream transpose, which is exact
    here because att is 16x16 block-diagonal.
  * MLP (C stages) of group g-1 is emitted after attention of group g so
    its matmuls fill the PE holes left by the softmax serial chain.
  * all matmul operands bf16 (PSUM accumulation fp32); the residual
    spine (x, y, out) and softmax stay fp32.
  * LN gains are folded into the following weight matrices on the host;
    LN biases fold into per-partition biases fused into PSUM-eviction
    activations; b_proj (+ folded v-bias) is added via a rank-1 matmul
    into PSUM. rstd = exp(-0.5 ln(var+eps)) keeps everything in the
    single ScalarE table set that also holds Exp/Identity/Relu.
"""

import sys

sys.path.insert(0, "/opt/trn_rl_repo")

import numpy as np
import ml_dtypes

import concourse.bass as bass
import concourse.tile as tile
from concourse import bacc, mybir
from concourse.bass_utils import run_bass_kernel_spmd

F32 = mybir.dt.float32
BF16 = mybir.dt.bfloat16
FP8 = mybir.dt.float8e4
DR = mybir.MatmulPerfMode.DoubleRow
AF = mybir.ActivationFunctionType
ALU = mybir.AluOpType

# fp8 weights are pre-scaled by WS so w~0.02 entries clear the e4m3
# subnormal range; the inverse is folded into each PSUM-eviction
# activation's `scale=`.
WS = 32.0

NCORES = 8
B, T, C, H, D = 4096, 16, 512, 8, 64
HD = H * D          # 512
M1 = 4 * C          # 2048
EPS = 1e-5
BL = B // NCORES    # 512 batch elems per core
NTOK_FULL = BL * T  # 8192 tokens per core
P = 128             # partitions
GT = 512            # tokens per group
KC = C // P         # 4 c-chunks
KM = M1 // P        # 16 hidden chunks

# K=64 row-tiled logits via tile_position (2 heads concurrent in the PE
# array). Fallback zero-pads k^T to full 128 partitions per head.
ROW_TILED_LOGITS = False


def emit_block(ctx, tc, outs, ins, ntok):
    """Emit the transformer-block program. outs/ins: dicts of DRAM APs."""
    nc = tc.nc
    x_d = ins["x"]
    wqk_d = ins["wqk"]          # [C, 2*HD] bf16 (g1-scaled)
    wv_d = ins["wv"]            # [C, HD]  bf16 (g1-scaled)
    wp_d = ins["wp"]            # [HD, C]  bf16
    w1_d = ins["w1"]            # [2*128, 2*M1]  fp8 paired (g2-scaled, xWS)
    w2_d = ins["w2"]            # [8*128, 2*C]   fp8 paired (xWS)
    bqk_d = ins["bqk"]          # [2*HD]   f32
    b1_d = ins["b1"]            # [M1]     f32
    b2_d = ins["b2"]            # [C]      f32
    bpe_d = ins["bpe"]          # [1, C]   bf16  (b_proj + ln1_b@wv@w_proj)
    mask_d = ins["mask"]        # [P, 4*P] bf16 additive causal mask (0/-300)
    ident_d = ins["ident"]      # [P, P]   bf16
    out_d = outs["out"]

    ngroups = ntok // GT
    assert ntok % GT == 0

    consts = ctx.enter_context(tc.tile_pool(name="consts", bufs=1))

    # --- resident weights/constants ---
    # fp8 weights (wqk/wv/w1) are stored "paired" for DoubleRow: tile m
    # holds C-chunks (2m, 2m+1) interleaved as [128, 2, cols].
    wqk = [consts.tile([P, 2 * 2 * HD], FP8, tag=f"wqk{m}", name=f"wqk{m}")
           for m in range(2)]
    wv = [consts.tile([P, 2 * HD], FP8, tag=f"wv{m}", name=f"wv{m}")
          for m in range(2)]
    wp = [consts.tile([P, C], BF16, tag=f"wp{k}", name=f"wp{k}")
          for k in range(KC)]
    w1 = [consts.tile([P, 2 * M1], FP8, tag=f"w1{m}", name=f"w1{m}")
          for m in range(2)]
    w2 = [consts.tile([P, C], BF16, tag=f"w2{k}", name=f"w2{k}")
          for k in range(KM)]
    for k in range(KC):
        nc.sync.dma_start(wp[k][:], wp_d[k * P:(k + 1) * P, :])
    for m in range(2):
        nc.sync.dma_start(wqk[m][:], wqk_d[m * P:(m + 1) * P, :])
        nc.sync.dma_start(wv[m][:], wv_d[m * P:(m + 1) * P, :])
        nc.sync.dma_start(w1[m][:], w1_d[m * P:(m + 1) * P, :])
    for k in range(KM):
        nc.sync.dma_start(w2[k][:], w2_d[k * P:(k + 1) * P, :])

    bqk = consts.tile([P, 2 * HD // P], F32, tag="bqk", name="bqk")     # [128, 8]
    b1t = consts.tile([P, KM], F32, tag="b1t", name="b1t")              # [128, 16]
    b2t = consts.tile([P, KC], F32, tag="b2t", name="b2t")              # [128, 4]
    nc.sync.dma_start(bqk[:], bqk_d.rearrange("(j p) -> p j", p=P))
    nc.sync.dma_start(b1t[:], b1_d.rearrange("(j p) -> p j", p=P))
    nc.sync.dma_start(b2t[:], b2_d.rearrange("(j p) -> p j", p=P))

    bpe = consts.tile([1, C], BF16, tag="bpe", name="bpe")
    nc.sync.dma_start(bpe[:], bpe_d[:, :])
    # [P, 4*P] bf16: 0 on block-diag causal positions, -300 elsewhere; added
    # into the logits PSUM via an accumulating identity-matmul so exp() masks
    # for free (exp(0.125 * -300) == 0 in bf16).
    mask = consts.tile([P, KC * P], BF16, tag="mask", name="mask")
    nc.sync.dma_start(mask[:], mask_d[:, :])
    ident = consts.tile([P, P], BF16, tag="ident", name="ident")
    nc.sync.dma_start(ident[:], ident_d[:, :])
    ones1 = consts.tile([1, P], BF16, tag="ones1", name="ones1")
    nc.vector.memset(ones1[:], 1.0)
    epst = consts.tile([P, 1], F32, tag="epst", name="epst")
    nc.vector.memset(epst[:], EPS)

    # --- working pools ---
    p_x = ctx.enter_context(tc.tile_pool(name="p_x", bufs=8))
    p_h = ctx.enter_context(tc.tile_pool(name="p_h", bufs=3))
    p_hT = ctx.enter_context(tc.tile_pool(name="p_hT", bufs=2))
    p_qk = ctx.enter_context(
        tc.tile_pool(name="p_qk", bufs=16 if ROW_TILED_LOGITS else 20))
    p_v = ctx.enter_context(tc.tile_pool(name="p_v", bufs=6))
    p_S = ctx.enter_context(tc.tile_pool(name="p_S", bufs=4))
    p_att = ctx.enter_context(tc.tile_pool(name="p_att", bufs=4))
    p_attT = ctx.enter_context(tc.tile_pool(name="p_attT", bufs=4))
    p_oT = ctx.enter_context(tc.tile_pool(name="p_oT", bufs=8))
    p_y = ctx.enter_context(tc.tile_pool(name="p_y", bufs=8))
    p_h2T = ctx.enter_context(tc.tile_pool(name="p_h2T", bufs=3))
    p_r = ctx.enter_context(tc.tile_pool(name="p_r", bufs=20))
    p_fT = ctx.enter_context(tc.tile_pool(name="p_fT", bufs=8))
    p_out = ctx.enter_context(tc.tile_pool(name="p_out", bufs=4))
    p_st = ctx.enter_context(tc.tile_pool(name="p_st", bufs=10))

    ps_mm = ctx.enter_context(tc.tile_pool(name="ps_mm", bufs=5, space="PSUM"))
    ps_t = ctx.enter_context(tc.tile_pool(name="ps_t", bufs=2, space="PSUM"))

    def layernorm(x_t, h_t):
        """h_t (bf16) = (x_t - mean) * rstd, per partition row."""
        st = p_st.tile([P, 6], F32, tag="bn", name="bn")
        mv = p_st.tile([P, 2], F32, tag="mv", name="mv")
        nc.vector.bn_stats(st[:], x_t[:])
        nc.vector.bn_aggr(mv[:], st[:])
        lnv = p_st.tile([P, 1], F32, tag="lnv", name="lnv")
        rstd = p_st.tile([P, 1], F32, tag="rstd", name="rstd")
        nc.scalar.activation(lnv[:], mv[:, 1:2], AF.Ln, bias=epst[:])
        nc.scalar.activation(rstd[:], lnv[:], AF.Exp, scale=-0.5)
        nc.vector.tensor_scalar(
            out=h_t[:], in0=x_t[:],
            scalar1=mv[:, 0:1], scalar2=rstd[:],
            op0=ALU.subtract, op1=ALU.mult,
        )

    def stage_a_dma(g):
        """Issue the x loads for group g (DMA engines are near idle)."""
        x_t = []
        for i in range(4):
            ti = 4 * g + i
            xt = p_x.tile([P, C], F32, tag="x", name="x")
            x_t.append(xt)
            nc.sync.dma_start(xt[:], x_d[ti * P:(ti + 1) * P, :])
        return x_t

    def stage_a_tile(x_t, hTg, i):
        """LN1 + feature-major transpose for one 128-token tile."""
        hTs = hTg[:].rearrange("p (c t) -> p c t", c=KC)
        ht = p_h.tile([P, C], BF16, tag="h", name="h")
        layernorm(x_t[i], ht)
        pst = ps_t.tile([P, KC * P], BF16, tag="pst", name="pst")
        for c in range(KC):
            nc.tensor.transpose(pst[:, c * P:(c + 1) * P],
                                ht[:, c * P:(c + 1) * P], ident[:])
        nc.vector.tensor_copy(hTs[:, :, i * P:(i + 1) * P], pst[:])

    def make_c_chunks(h2Tg, y_t, g):
        """Deferred MLP of group g as a list of emission closures so the
        matmuls can be interleaved into the next group's softmax holes."""
        r_t = []
        fT = []
        chunks = []

        h2T3 = h2Tg[:].rearrange("p (c t) -> p c t", c=KC)

        def c1(j):
            ps = ps_mm.tile([P, GT], F32, tag="mm", name="mm")
            for m in range(2):
                w1s = w1[m][:].rearrange("p (c j) -> p c j", c=2)
                nc.tensor.matmul(
                    ps[:], w1s[:, :, j * P:(j + 1) * P],
                    h2T3[:, 2 * m:2 * m + 2, :],
                    start=(m == 0), stop=(m == 1), perf_mode=DR,
                )
            rt = p_r.tile([P, GT], BF16, tag="r", name="r")
            r_t.append(rt)
            nc.scalar.activation(rt[:], ps[:], AF.Relu,
                                 bias=b1t[:, j:j + 1], scale=1.0 / WS)

        def c2(q):
            ps = ps_mm.tile([P, GT], F32, tag="mm", name="mm")
            for k in range(KM):
                nc.tensor.matmul(
                    ps[:], w2[k][:, q * P:(q + 1) * P], r_t[k][:],
                    start=(k == 0), stop=(k == KM - 1),
                )
            ft = p_fT.tile([P, GT], BF16, tag="fT", name="fT")
            fT.append(ft)
            nc.scalar.activation(ft[:], ps[:], AF.Identity, bias=b2t[:, q:q + 1])

        def c3(i):
            ti = 4 * g + i
            ot = p_out.tile([P, C], F32, tag="out", name="out")
            pst = ps_t.tile([P, KC * P], BF16, tag="pst", name="pst")
            for q in range(KC):
                nc.tensor.transpose(pst[:, q * P:(q + 1) * P],
                                    fT[q][:, i * P:(i + 1) * P], ident[:])
            nc.vector.tensor_tensor(out=ot[:], in0=pst[:], in1=y_t[i][:],
                                    op=ALU.add)
            nc.sync.dma_start(out_d[ti * P:(ti + 1) * P, :], ot[:])

        from functools import partial
        for j in range(KM):
            chunks.append(partial(c1, j))
        for q in range(KC):
            chunks.append(partial(c2, q))
        for i in range(4):
            chunks.append(partial(c3, i))
        return chunks

    pend_c = []

    def drain_c(n):
        for _ in range(min(n, len(pend_c))):
            pend_c.pop(0)()

    # prologue: group 0 input prep
    x_t = stage_a_dma(0)
    hTg = p_hT.tile([P, KC * GT], FP8, tag="hT", name="hT")
    for i in range(4):
        stage_a_tile(x_t, hTg, i)

    for g in range(ngroups):
        y_t = []
        hT3 = hTg[:].rearrange("p (c t) -> p c t", c=KC)
        if g + 1 < ngroups:
            x_t_next = stage_a_dma(g + 1)
            hTg_next = p_hT.tile([P, KC * GT], FP8, tag="hT", name="hT")

        # ---- stage B1: q^T, k^T feature-major (weight stationary) ----
        qT = [p_qk.tile([P, GT], BF16, tag="qk", name="qk") for _ in range(KC)]
        if ROW_TILED_LOGITS:
            kT = [p_qk.tile([P, GT], BF16, tag="qk", name="qk")
                  for _ in range(KC)]
        else:
            # zero-padded per-head k^T (other head's rows zeroed) so logits
            # matmuls use full-128-partition operands.
            kTe = [p_qk.tile([P, GT], BF16, tag="qk", name="qk")
                   for _ in range(KC)]
            kTo = [p_qk.tile([P, GT], BF16, tag="qk", name="qk")
                   for _ in range(KC)]
        for j in range(8):
            ps = ps_mm.tile([P, GT], F32, tag="mm", name="mm")
            for m in range(2):
                wqks = wqk[m][:].rearrange("p (c j) -> p c j", c=2)
                nc.tensor.matmul(
                    ps[:], wqks[:, :, j * P:(j + 1) * P],
                    hT3[:, 2 * m:2 * m + 2, :],
                    start=(m == 0), stop=(m == 1), perf_mode=DR,
                )
            if j < 4 or ROW_TILED_LOGITS:
                dst = qT[j] if j < 4 else kT[j - 4]
                nc.scalar.activation(dst[:], ps[:], AF.Identity,
                                     bias=bqk[:, j:j + 1], scale=1.0 / WS)
            else:
                e, o_ = kTe[j - 4], kTo[j - 4]
                nc.gpsimd.memset(e[64:128, :], 0.0)
                nc.gpsimd.memset(o_[0:64, :], 0.0)
                nc.scalar.activation(e[0:64, :], ps[0:64, :], AF.Identity,
                                     bias=bqk[0:64, j:j + 1], scale=1.0 / WS)
                nc.scalar.activation(o_[64:128, :], ps[64:128, :], AF.Identity,
                                     bias=bqk[64:128, j:j + 1], scale=1.0 / WS)

        # ---- stage B2: v token-major (h^T stationary) ----
        v_t = []
        for i in range(4):
            ps = ps_mm.tile([P, HD], F32, tag="mm", name="mm")
            for m in range(2):
                wvs = wv[m][:].rearrange("p (c d) -> p c d", c=2)
                nc.tensor.matmul(
                    ps[:], hT3[:, 2 * m:2 * m + 2, i * P:(i + 1) * P],
                    wvs[:],
                    start=(m == 0), stop=(m == 1), perf_mode=DR,
                )
            vt = p_v.tile([P, HD], BF16, tag="v", name="v")
            v_t.append(vt)
            nc.scalar.activation(vt[:], ps[:], AF.Identity, scale=1.0 / WS)

        # ---- stage B3: attention per token tile ----
        oTa = []
        for i in range(4):
            sl = slice(i * P, (i + 1) * P)
            S = p_S.tile([P, H * P], BF16, tag="S", name="S")
            rs = p_st.tile([P, H], F32, tag="rs", name="rs")
            for half in range(2):
                ps_l = ps_mm.tile([P, C], F32, tag="mm", name="mm")
                for hh in range(4):
                    h = half * 4 + hh
                    hp = h // 2
                    ho = (h % 2) * 64
                    if ROW_TILED_LOGITS:
                        nc.tensor.matmul(
                            ps_l[:, hh * P:(hh + 1) * P],
                            qT[hp][ho:ho + 64, sl], kT[hp][ho:ho + 64, sl],
                            start=(hh == 0), stop=False,
                            tile_position=(ho, 0),
                        )
                    else:
                        kk = kTe[hp] if h % 2 == 0 else kTo[hp]
                        nc.tensor.matmul(
                            ps_l[:, hh * P:(hh + 1) * P],
                            qT[hp][:, sl], kk[:, sl],
                            start=(hh == 0), stop=False,
                        )
                # causal block-diag mask: accumulate -300 at disallowed spots
                nc.tensor.matmul(ps_l[:], ident[:], mask[:],
                                 start=False, stop=True)
                # per-head exp so the free ScalarE accumulator yields the
                # softmax row sums (masked entries contribute exp(-37)=0)
                for hh in range(4):
                    h = half * 4 + hh
                    nc.scalar.activation(
                        S[:, h * P:(h + 1) * P], ps_l[:, hh * P:(hh + 1) * P],
                        AF.Exp, scale=float(D) ** -0.5,
                        accum_out=rs[:, h:h + 1])
            att = p_att.tile([P, H * P], BF16, tag="att", name="att")
            rr = p_st.tile([P, H], F32, tag="rr", name="rr")
            nc.vector.reciprocal(rr[:], rs[:])
            for h in range(H):
                hs = slice(h * P, (h + 1) * P)
                nc.vector.tensor_scalar_mul(att[:, hs], S[:, hs], rr[:, h:h + 1])
            # att is 16x16 block-diagonal, so the DVE 32x32 block transpose
            # equals the full per-head transpose.
            attT = p_attT.tile([P, H * P], BF16, tag="attT", name="attT")
            nc.vector.transpose(attT[:], att[:])
            # fill the PE idle while this tile's softmax chain runs on
            # ScalarE/VectorE: MLP matmuls of the previous group + input
            # prep (LN1 + transposes) of the next group.
            drain_c(3)
            if g + 1 < ngroups:
                stage_a_tile(x_t_next, hTg_next, i)
            # o^T pieces: lhsT = v slice [128,(b,s)] x [64 d] -> out [64, 128]
            ps_o = ps_mm.tile([P, C], F32, tag="mm", name="mm")
            for h in range(H):
                nc.tensor.matmul(
                    ps_o[(h % 2) * 64:(h % 2) * 64 + 64,
                         (h // 2) * P:(h // 2) * P + P],
                    v_t[i][:, h * 64:(h + 1) * 64],
                    attT[:, h * P:(h + 1) * P],
                    start=True, stop=True,
                )
            ot = p_oT.tile([P, C], BF16, tag="oT", name="oT")
            oTa.append(ot)
            nc.scalar.activation(ot[:], ps_o[:], AF.Identity)

        # ---- stage B4: proj + residual -> y ----
        for i in range(4):
            ps = ps_mm.tile([P, C], F32, tag="mm", name="mm")
            for p in range(KC):
                nc.tensor.matmul(
                    ps[:], oTa[i][:, p * P:(p + 1) * P], wp[p][:],
                    start=(p == 0), stop=False,
                )
            nc.tensor.matmul(ps[:], ones1[:], bpe[:], start=False, stop=True)
            yt = p_y.tile([P, C], F32, tag="y", name="y")
            y_t.append(yt)
            nc.vector.tensor_tensor(out=yt[:], in0=ps[:], in1=x_t[i][:], op=ALU.add)
        drain_c(8)

        # ---- stage B5: LN2 + transpose ----
        h2Tg = p_h2T.tile([P, KC * GT], FP8, tag="h2T", name="h2T")
        h2Ts = h2Tg[:].rearrange("p (c t) -> p c t", c=KC)
        for i in range(4):
            ht2 = p_h.tile([P, C], BF16, tag="h", name="h")
            layernorm(y_t[i], ht2)
            pst = ps_t.tile([P, KC * P], BF16, tag="pst", name="pst")
            for c in range(KC):
                nc.tensor.transpose(pst[:, c * P:(c + 1) * P],
                                    ht2[:, c * P:(c + 1) * P], ident[:])
            nc.vector.tensor_copy(h2Ts[:, :, i * P:(i + 1) * P], pst[:])
        drain_c(len(pend_c))

        pend_c = make_c_chunks(h2Tg, y_t, g)
        if g + 1 < ngroups:
            x_t, hTg = x_t_next, hTg_next

    drain_c(len(pend_c))


def _fp8_pair(w, ncols):
    """[K, ncols] f32 -> [K//256, 128, 2, ncols] fp8 paired chunks, flattened
    to [K//2, 2*ncols] for a contiguous per-tile DMA."""
    fp8 = ml_dtypes.float8_e4m3
    K = w.shape[0]
    npair = K // 256
    wp = w.reshape(npair, 2, P, ncols).transpose(0, 2, 1, 3)  # [m, p, c, cols]
    wp = np.clip(wp * WS, -240, 240)
    return np.ascontiguousarray(wp.reshape(npair * P, 2 * ncols)).astype(fp8)


def preprocess(inputs):
    """Host-side weight folding. Returns dict of extra device arrays."""
    f32 = np.float32
    bf16 = ml_dtypes.bfloat16
    g1 = np.asarray(inputs["ln1_g"], f32)
    b1n = np.asarray(inputs["ln1_b"], f32)
    g2 = np.asarray(inputs["ln2_g"], f32)
    b2n = np.asarray(inputs["ln2_b"], f32)
    wq = np.asarray(inputs["wq"], f32).transpose(1, 0, 2).reshape(C, HD)
    wk = np.asarray(inputs["wk"], f32).transpose(1, 0, 2).reshape(C, HD)
    wv = np.asarray(inputs["wv"], f32).transpose(1, 0, 2).reshape(C, HD)
    w_proj = np.asarray(inputs["w_proj"], f32)
    b_proj = np.asarray(inputs["b_proj"], f32)
    w1 = np.asarray(inputs["w1"], f32)
    b1 = np.asarray(inputs["b1"], f32)
    w2 = np.asarray(inputs["w2"], f32)
    b2 = np.asarray(inputs["b2"], f32)

    wqg = wq * g1[:, None]
    wkg = wk * g1[:, None]
    wvg = wv * g1[:, None]
    w1g = w1 * g2[:, None]

    wqk = np.concatenate([wqg, wkg], axis=1)          # [C, 1024]
    bqk = b1n @ wqk                                   # [1024]
    bias_v = b1n @ wvg                                # [512]
    bpe = (bias_v @ w_proj + b_proj)[None, :]         # [1, 512]
    bias1 = b2n @ w1g + b1                            # [2048]

    # additive mask: 0 on allowed (block-diag causal), -300 elsewhere
    mask = np.full((P, P), -300.0, f32)
    tril = np.tril(np.ones((T, T), bool))
    for b in range(P // T):
        blk = mask[b * T:(b + 1) * T, b * T:(b + 1) * T]
        blk[tril] = 0.0

    return {
        "wqk": _fp8_pair(wqk, 2 * HD),
        "wv": _fp8_pair(wvg, HD),
        "wp": w_proj.astype(bf16),
        "w1": _fp8_pair(w1g, M1),
        "w2": w2.astype(bf16),
        "bqk": bqk.astype(f32),
        "b1": bias1.astype(f32),
        "b2": b2.astype(f32),
        "bpe": bpe.astype(bf16),
        "mask": np.tile(mask, (1, KC)).astype(bf16),
        "ident": np.eye(P, dtype=bf16),
    }


def _patch_act_tables():
    """Make every activation func we use resolve to the single table set
    `natural_log_exp_and_others` (it contains Ln, Exp, Identity and Relu),
    so bacc's table-load pass emits one load instead of thrashing between
    `natural_log` and `exp_and_others` (~2.7us per switch). Indices of the
    table list are preserved so act_func_set_ids stay valid."""
    import concourse.bacc as _bacc_mod
    import concourse.hw_specs as _hw
    if getattr(_bacc_mod, "_ant_act_tables_patched", False):
        return
    _orig = _hw.get_activation_tables
    ours = {AF.Ln, AF.Exp, AF.Identity, AF.Relu, AF.Copy}

    def patched(arch):
        tables = _orig(arch)
        out = {}
        for name, funcs in tables.items():
            if name == "natural_log_exp_and_others":
                out[name] = funcs
            else:
                out[name] = funcs - ours
        return out

    _bacc_mod.get_activation_tables = patched
    _bacc_mod._ant_act_tables_patched = True


def build(ntok=NTOK_FULL):
    """Build the Bass program; returns nc."""
    from contextlib import ExitStack

    _patch_act_tables()
    nc = bacc.Bacc("TRN2", target_bir_lowering=False, debug=False,
                   enable_asserts=False, num_devices=NCORES)
    ins = {
        "x": nc.dram_tensor("x", [ntok, C], F32, kind="ExternalInput").ap(),
        "wqk": nc.dram_tensor("wqk", [2 * P, 2 * 2 * HD], FP8,
                              kind="ExternalInput").ap(),
        "wv": nc.dram_tensor("wv", [2 * P, 2 * HD], FP8,
                             kind="ExternalInput").ap(),
        "wp": nc.dram_tensor("wp", [HD, C], BF16, kind="ExternalInput").ap(),
        "w1": nc.dram_tensor("w1", [2 * P, 2 * M1], FP8,
                             kind="ExternalInput").ap(),
        "w2": nc.dram_tensor("w2", [M1, C], BF16, kind="ExternalInput").ap(),
        "bqk": nc.dram_tensor("bqk", [2 * HD], F32, kind="ExternalInput").ap(),
        "b1": nc.dram_tensor("b1", [M1], F32, kind="ExternalInput").ap(),
        "b2": nc.dram_tensor("b2", [C], F32, kind="ExternalInput").ap(),
        "bpe": nc.dram_tensor("bpe", [1, C], BF16, kind="ExternalInput").ap(),
        "mask": nc.dram_tensor("mask", [P, KC * P], BF16, kind="ExternalInput").ap(),
        "ident": nc.dram_tensor("ident", [P, P], BF16, kind="ExternalInput").ap(),
    }
    outs = {
        "out": nc.dram_tensor("out", [ntok, C], F32, kind="ExternalOutput").ap(),
    }
    with ExitStack() as ctx:
        tc = ctx.enter_context(tile.TileContext(nc))
        emit_block(ctx, tc, outs, ins, ntok)
    nc.finalize()
    return nc


def kernel(**inputs):
    x = np.ascontiguousarray(np.asarray(inputs["x"], np.float32))
    consts = preprocess(inputs)
    nc = build(NTOK_FULL)
    xs = x.reshape(NCORES, NTOK_FULL, C)
    in_maps = [dict(consts, x=np.ascontiguousarray(xs[c])) for c in range(NCORES)]
    res = run_bass_kernel_spmd(nc, in_maps, core_ids=list(range(NCORES)))
    out = np.stack([res.results[c]["out"] for c in range(NCORES)], axis=0)
    return out.reshape(B, T, C).astype(np.float32)


if __name__ == "__main__":
    rng = np.random.default_rng(0)
    fake = {
        "x": rng.standard_normal((B, T, C), dtype=np.float32),
        "ln1_g": np.ones(C, np.float32), "ln1_b": np.zeros(C, np.float32),
        "wq": rng.standard_normal((H, C, D), dtype=np.float32) * 0.02,
        "wk": rng.standard_normal((H, C, D), dtype=np.float32) * 0.02,
        "wv": rng.standard_normal((H, C, D), dtype=np.float32) * 0.02,
        "w_proj": rng.standard_normal((HD, C), dtype=np.float32) * 0.02,
        "b_proj": np.zeros(C, np.float32),
        "ln2_g": np.ones(C, np.float32), "ln2_b": np.zeros(C, np.float32),
        "w1": rng.standard_normal((C, M1), dtype=np.float32) * 0.02,
        "b1": np.zeros(M1, np.float32),
        "w2": rng.standard_normal((M1, C), dtype=np.float32) * 0.02,
        "b2": np.zeros(C, np.float32),
    }
    out = kernel(**fake)
    print("kernel ran, out shape", out.shape)


# revision 53
# speedup vs baseline: 1.0607x; 1.0607x over previous
"""Trainium2 Bass/Tile kernel for a dense transformer block.

Math (per batch element b, T=16 tokens, C=512, H=8 heads, D=64):
    h  = LN(x; ln1_g, ln1_b)
    q,k,v = per-head projections of h
    att = causal-softmax(q k^T / sqrt(D)); o = att v (heads concatenated)
    y  = o @ w_proj + b_proj + x
    f  = relu(LN(y; ln2_g, ln2_b) @ w1 + b1) @ w2 + b2
    out = f + y

Distribution: pure data parallel over the batch dim (4096) across 8
NeuronCores; weights replicated; no collectives.

Device layout strategy (per core, 512 batch elems = 8192 tokens):
  * tokens-on-partitions for x, LN, v, y, out (128 tokens/tile, 64 tiles)
  * feature-major (C-on-partitions) for matmul moving operands, obtained
    with PE transposes; q^T/k^T produced feature-major directly by
    weight-stationary matmuls so per-head [64 x 128] slices feed the
    attention matmuls with no extra copies.
  * attention per 128-token tile (8 whole batch elems): per-head
    [128, 128] logits via K=64 row-tiled matmuls (tile_position packs two
    heads into disjoint row-halves of the PE array so they run
    concurrently); exp on ScalarE; block-diag causal mask + row sums on
    VectorE; att^T via the DVE 32x32 stream transpose, which is exact
    here because att is 16x16 block-diagonal.
  * MLP (C stages) of group g-1 is emitted after attention of group g so
    its matmuls fill the PE holes left by the softmax serial chain.
  * all matmul operands bf16 (PSUM accumulation fp32); the residual
    spine (x, y, out) and softmax stay fp32.
  * LN gains are folded into the following weight matrices on the host;
    LN biases fold into per-partition biases fused into PSUM-eviction
    activations; b_proj (+ folded v-bias) is added via a rank-1 matmul
    into PSUM. rstd = exp(-0.5 ln(var+eps)) keeps everything in the
    single ScalarE table set that also holds Exp/Identity/Relu.
"""

import sys

sys.path.insert(0, "/opt/trn_rl_repo")

import numpy as np
import ml_dtypes

import concourse.bass as bass
import concourse.tile as tile
from concourse import bacc, mybir
from concourse.bass_utils import run_bass_kernel_spmd

F32 = mybir.dt.float32
BF16 = mybir.dt.bfloat16
FP8 = mybir.dt.float8e4
DR = mybir.MatmulPerfMode.DoubleRow
AF = mybir.ActivationFunctionType
ALU = mybir.AluOpType

# fp8 weights are pre-scaled by WS so w~0.02 entries clear the e4m3
# subnormal range; the inverse is folded into each PSUM-eviction
# activation's `scale=`.
WS = 32.0

NCORES = 8
B, T, C, H, D = 4096, 16, 512, 8, 64
HD = H * D          # 512
M1 = 4 * C          # 2048
EPS = 1e-5
BL = B // NCORES    # 512 batch elems per core
NTOK_FULL = BL * T  # 8192 tokens per core
P = 128             # partitions
GT = 512            # tokens per group
KC = C // P         # 4 c-chunks
KM = M1 // P        # 16 hidden chunks

# K=64 row-tiled logits via tile_position (2 heads concurrent in the PE
# array). Fallback zero-pads k^T to full 128 partitions per head.
ROW_TILED_LOGITS = False


def emit_block(ctx, tc, outs, ins, ntok):
    """Emit the transformer-block program. outs/ins: dicts of DRAM APs."""
    nc = tc.nc
    x_d = ins["x"]
    wqk_d = ins["wqk"]          # [C, 2*HD] bf16 (g1-scaled)
    wv_d = ins["wv"]            # [C, HD]  bf16 (g1-scaled)
    wp_d = ins["wp"]            # [HD, C]  bf16
    w1_d = ins["w1"]            # [2*128, 2*M1]  fp8 paired (g2-scaled, xWS)
    w2_d = ins["w2"]            # [8*128, 2*C]   fp8 paired (xWS)
    bqk_d = ins["bqk"]          # [2*HD]   f32
    b1_d = ins["b1"]            # [M1]     f32
    b2_d = ins["b2"]            # [C]      f32
    bpe_d = ins["bpe"]          # [1, C]   bf16  (b_proj + ln1_b@wv@w_proj)
    mask_d = ins["mask"]        # [P, 4*P] bf16 additive causal mask (0/-300)
    ident_d = ins["ident"]      # [P, P]   bf16
    out_d = outs["out"]

    ngroups = ntok // GT
    assert ntok % GT == 0

    consts = ctx.enter_context(tc.tile_pool(name="consts", bufs=1))

    # --- resident weights/constants ---
    # fp8 weights (wqk/wv/w1) are stored "paired" for DoubleRow: tile m
    # holds C-chunks (2m, 2m+1) interleaved as [128, 2, cols].
    wqk = [consts.tile([P, 2 * 2 * HD], FP8, tag=f"wqk{m}", name=f"wqk{m}")
           for m in range(2)]
    wv = [consts.tile([P, 2 * HD], FP8, tag=f"wv{m}", name=f"wv{m}")
          for m in range(2)]
    wp = [consts.tile([P, C], BF16, tag=f"wp{k}", name=f"wp{k}")
          for k in range(KC)]
    w1 = [consts.tile([P, 2 * M1], FP8, tag=f"w1{m}", name=f"w1{m}")
          for m in range(2)]
    w2 = [consts.tile([P, C], BF16, tag=f"w2{k}", name=f"w2{k}")
          for k in range(KM)]
    for k in range(KC):
        nc.sync.dma_start(wp[k][:], wp_d[k * P:(k + 1) * P, :])
    for m in range(2):
        nc.sync.dma_start(wqk[m][:], wqk_d[m * P:(m + 1) * P, :])
        nc.sync.dma_start(wv[m][:], wv_d[m * P:(m + 1) * P, :])
        nc.sync.dma_start(w1[m][:], w1_d[m * P:(m + 1) * P, :])
    for k in range(KM):
        nc.sync.dma_start(w2[k][:], w2_d[k * P:(k + 1) * P, :])

    bqk = consts.tile([P, 2 * HD // P], F32, tag="bqk", name="bqk")     # [128, 8]
    b1t = consts.tile([P, KM], F32, tag="b1t", name="b1t")              # [128, 16]
    b2t = consts.tile([P, KC], F32, tag="b2t", name="b2t")              # [128, 4]
    nc.sync.dma_start(bqk[:], bqk_d.rearrange("(j p) -> p j", p=P))
    nc.sync.dma_start(b1t[:], b1_d.rearrange("(j p) -> p j", p=P))
    nc.sync.dma_start(b2t[:], b2_d.rearrange("(j p) -> p j", p=P))

    bpe = consts.tile([1, C], BF16, tag="bpe", name="bpe")
    nc.sync.dma_start(bpe[:], bpe_d[:, :])
    # [P, 4*P] bf16: 0 on block-diag causal positions, -300 elsewhere; added
    # into the logits PSUM via an accumulating identity-matmul so exp() masks
    # for free (exp(0.125 * -300) == 0 in bf16).
    mask = consts.tile([P, KC * P], BF16, tag="mask", name="mask")
    nc.sync.dma_start(mask[:], mask_d[:, :])
    ident = consts.tile([P, P], BF16, tag="ident", name="ident")
    nc.sync.dma_start(ident[:], ident_d[:, :])
    ones1 = consts.tile([1, P], BF16, tag="ones1", name="ones1")
    nc.vector.memset(ones1[:], 1.0)
    epst = consts.tile([P, 1], F32, tag="epst", name="epst")
    nc.vector.memset(epst[:], EPS)

    # --- working pools ---
    p_x = ctx.enter_context(tc.tile_pool(name="p_x", bufs=8))
    p_h = ctx.enter_context(tc.tile_pool(name="p_h", bufs=3))
    p_hT = ctx.enter_context(tc.tile_pool(name="p_hT", bufs=2))
    p_qk = ctx.enter_context(
        tc.tile_pool(name="p_qk", bufs=16 if ROW_TILED_LOGITS else 20))
    p_v = ctx.enter_context(tc.tile_pool(name="p_v", bufs=6))
    p_S = ctx.enter_context(tc.tile_pool(name="p_S", bufs=4))
    p_att = ctx.enter_context(tc.tile_pool(name="p_att", bufs=4))
    p_attT = ctx.enter_context(tc.tile_pool(name="p_attT", bufs=4))
    p_oT = ctx.enter_context(tc.tile_pool(name="p_oT", bufs=8))
    p_y = ctx.enter_context(tc.tile_pool(name="p_y", bufs=8))
    p_h2T = ctx.enter_context(tc.tile_pool(name="p_h2T", bufs=3))
    p_r = ctx.enter_context(tc.tile_pool(name="p_r", bufs=20))
    p_fT = ctx.enter_context(tc.tile_pool(name="p_fT", bufs=8))
    p_out = ctx.enter_context(tc.tile_pool(name="p_out", bufs=4))
    p_st = ctx.enter_context(tc.tile_pool(name="p_st", bufs=10))

    ps_mm = ctx.enter_context(tc.tile_pool(name="ps_mm", bufs=5, space="PSUM"))
    ps_t = ctx.enter_context(tc.tile_pool(name="ps_t", bufs=2, space="PSUM"))

    def layernorm(x_t, h_t):
        """h_t (bf16) = (x_t - mean) * rstd, per partition row."""
        st = p_st.tile([P, 6], F32, tag="bn", name="bn")
        mv = p_st.tile([P, 2], F32, tag="mv", name="mv")
        nc.vector.bn_stats(st[:], x_t[:])
        nc.vector.bn_aggr(mv[:], st[:])
        lnv = p_st.tile([P, 1], F32, tag="lnv", name="lnv")
        rstd = p_st.tile([P, 1], F32, tag="rstd", name="rstd")
        nc.scalar.activation(lnv[:], mv[:, 1:2], AF.Ln, bias=epst[:])
        nc.scalar.activation(rstd[:], lnv[:], AF.Exp, scale=-0.5)
        nc.vector.tensor_scalar(
            out=h_t[:], in0=x_t[:],
            scalar1=mv[:, 0:1], scalar2=rstd[:],
            op0=ALU.subtract, op1=ALU.mult,
        )

    def stage_a_dma(g):
        """Issue the x loads for group g (DMA engines are near idle)."""
        x_t = []
        for i in range(4):
            ti = 4 * g + i
            xt = p_x.tile([P, C], F32, tag="x", name="x")
            x_t.append(xt)
            nc.sync.dma_start(xt[:], x_d[ti * P:(ti + 1) * P, :])
        return x_t

    def stage_a_tile(x_t, hTg, i):
        """LN1 + feature-major transpose for one 128-token tile."""
        hTs = hTg[:].rearrange("p (c t) -> p c t", c=KC)
        ht = p_h.tile([P, C], BF16, tag="h", name="h")
        layernorm(x_t[i], ht)
        pst = ps_t.tile([P, KC * P], BF16, tag="pst", name="pst")
        for c in range(KC):
            nc.tensor.transpose(pst[:, c * P:(c + 1) * P],
                                ht[:, c * P:(c + 1) * P], ident[:])
        nc.vector.tensor_copy(hTs[:, :, i * P:(i + 1) * P], pst[:])

    def make_c_chunks(h2Tg, y_t, g):
        """Deferred MLP of group g as a list of emission closures so the
        matmuls can be interleaved into the next group's softmax holes."""
        r_t = []
        fT = []
        chunks = []

        h2T3 = h2Tg[:].rearrange("p (c t) -> p c t", c=KC)

        def c1(j):
            ps = ps_mm.tile([P, GT], F32, tag="mm", name="mm")
            for m in range(2):
                w1s = w1[m][:].rearrange("p (c j) -> p c j", c=2)
                nc.tensor.matmul(
                    ps[:], w1s[:, :, j * P:(j + 1) * P],
                    h2T3[:, 2 * m:2 * m + 2, :],
                    start=(m == 0), stop=(m == 1), perf_mode=DR,
                )
            rt = p_r.tile([P, GT], BF16, tag="r", name="r")
            r_t.append(rt)
            # alternate the relu eviction between ScalarE and VectorE; r is
            # stored scaled by WS (b1 is pre-scaled by WS, w2 pre-divided).
            if j % 2 == 0:
                nc.scalar.activation(rt[:], ps[:], AF.Relu,
                                     bias=b1t[:, j:j + 1])
            else:
                nc.vector.tensor_scalar(
                    out=rt[:], in0=ps[:],
                    scalar1=b1t[:, j:j + 1], scalar2=0.0,
                    op0=ALU.add, op1=ALU.max,
                )

        def c2(q):
            ps = ps_mm.tile([P, GT], F32, tag="mm", name="mm")
            for k in range(KM):
                nc.tensor.matmul(
                    ps[:], w2[k][:, q * P:(q + 1) * P], r_t[k][:],
                    start=(k == 0), stop=(k == KM - 1),
                )
            ft = p_fT.tile([P, GT], BF16, tag="fT", name="fT")
            fT.append(ft)
            nc.scalar.activation(ft[:], ps[:], AF.Identity, bias=b2t[:, q:q + 1])

        def c3(i):
            ti = 4 * g + i
            ot = p_out.tile([P, C], F32, tag="out", name="out")
            pst = ps_t.tile([P, KC * P], BF16, tag="pst", name="pst")
            for q in range(KC):
                nc.tensor.transpose(pst[:, q * P:(q + 1) * P],
                                    fT[q][:, i * P:(i + 1) * P], ident[:])
            nc.vector.tensor_tensor(out=ot[:], in0=pst[:], in1=y_t[i][:],
                                    op=ALU.add)
            nc.sync.dma_start(out_d[ti * P:(ti + 1) * P, :], ot[:])

        from functools import partial
        for j in range(KM):
            chunks.append(partial(c1, j))
        for q in range(KC):
            chunks.append(partial(c2, q))
        for i in range(4):
            chunks.append(partial(c3, i))
        return chunks

    pend_c = []

    def drain_c(n):
        for _ in range(min(n, len(pend_c))):
            pend_c.pop(0)()

    # prologue: group 0 input prep
    x_t = stage_a_dma(0)
    hTg = p_hT.tile([P, KC * GT], FP8, tag="hT", name="hT")
    for i in range(4):
        stage_a_tile(x_t, hTg, i)

    for g in range(ngroups):
        y_t = []
        hT3 = hTg[:].rearrange("p (c t) -> p c t", c=KC)
        if g + 1 < ngroups:
            x_t_next = stage_a_dma(g + 1)
            hTg_next = p_hT.tile([P, KC * GT], FP8, tag="hT", name="hT")

        # ---- stage B1: q^T, k^T feature-major (weight stationary) ----
        qT = [p_qk.tile([P, GT], BF16, tag="qk", name="qk") for _ in range(KC)]
        if ROW_TILED_LOGITS:
            kT = [p_qk.tile([P, GT], BF16, tag="qk", name="qk")
                  for _ in range(KC)]
        else:
            # zero-padded per-head k^T (other head's rows zeroed) so logits
            # matmuls use full-128-partition operands.
            kTe = [p_qk.tile([P, GT], BF16, tag="qk", name="qk")
                   for _ in range(KC)]
            kTo = [p_qk.tile([P, GT], BF16, tag="qk", name="qk")
                   for _ in range(KC)]
        for j in range(8):
            ps = ps_mm.tile([P, GT], F32, tag="mm", name="mm")
            for m in range(2):
                wqks = wqk[m][:].rearrange("p (c j) -> p c j", c=2)
                nc.tensor.matmul(
                    ps[:], wqks[:, :, j * P:(j + 1) * P],
                    hT3[:, 2 * m:2 * m + 2, :],
                    start=(m == 0), stop=(m == 1), perf_mode=DR,
                )
            if j < 4 or ROW_TILED_LOGITS:
                dst = qT[j] if j < 4 else kT[j - 4]
                nc.scalar.activation(dst[:], ps[:], AF.Identity,
                                     bias=bqk[:, j:j + 1], scale=1.0 / WS)
            else:
                e, o_ = kTe[j - 4], kTo[j - 4]
                nc.gpsimd.memset(e[64:128, :], 0.0)
                nc.gpsimd.memset(o_[0:64, :], 0.0)
                nc.scalar.activation(e[0:64, :], ps[0:64, :], AF.Identity,
                                     bias=bqk[0:64, j:j + 1], scale=1.0 / WS)
                nc.scalar.activation(o_[64:128, :], ps[64:128, :], AF.Identity,
                                     bias=bqk[64:128, j:j + 1], scale=1.0 / WS)

        # ---- stage B2: v token-major (h^T stationary) ----
        v_t = []
        for i in range(4):
            ps = ps_mm.tile([P, HD], F32, tag="mm", name="mm")
            for m in range(2):
                wvs = wv[m][:].rearrange("p (c d) -> p c d", c=2)
                nc.tensor.matmul(
                    ps[:], hT3[:, 2 * m:2 * m + 2, i * P:(i + 1) * P],
                    wvs[:],
                    start=(m == 0), stop=(m == 1), perf_mode=DR,
                )
            vt = p_v.tile([P, HD], BF16, tag="v", name="v")
            v_t.append(vt)
            nc.scalar.activation(vt[:], ps[:], AF.Identity, scale=1.0 / WS)

        # ---- stage B3: attention per token tile ----
        oTa = []
        for i in range(4):
            sl = slice(i * P, (i + 1) * P)
            S = p_S.tile([P, H * P], BF16, tag="S", name="S")
            rs = p_st.tile([P, H], F32, tag="rs", name="rs")
            for half in range(2):
                ps_l = ps_mm.tile([P, C], F32, tag="mm", name="mm")
                for hh in range(4):
                    h = half * 4 + hh
                    hp = h // 2
                    ho = (h % 2) * 64
                    if ROW_TILED_LOGITS:
                        nc.tensor.matmul(
                            ps_l[:, hh * P:(hh + 1) * P],
                            qT[hp][ho:ho + 64, sl], kT[hp][ho:ho + 64, sl],
                            start=(hh == 0), stop=False,
                            tile_position=(ho, 0),
                        )
                    else:
                        kk = kTe[hp] if h % 2 == 0 else kTo[hp]
                        nc.tensor.matmul(
                            ps_l[:, hh * P:(hh + 1) * P],
                            qT[hp][:, sl], kk[:, sl],
                            start=(hh == 0), stop=False,
                        )
                # causal block-diag mask: accumulate -300 at disallowed spots
                nc.tensor.matmul(ps_l[:], ident[:], mask[:],
                                 start=False, stop=True)
                nc.scalar.activation(S[:, half * C:(half + 1) * C], ps_l[:],
                                     AF.Exp, scale=float(D) ** -0.5)
            att = p_att.tile([P, H * P], BF16, tag="att", name="att")
            nc.vector.tensor_reduce(
                out=rs[:],
                in_=S[:].rearrange("p (h s) -> p h s", h=H),
                axis=mybir.AxisListType.X, op=ALU.add,
            )
            rr = p_st.tile([P, H], F32, tag="rr", name="rr")
            nc.vector.reciprocal(rr[:], rs[:])
            for h in range(H):
                hs = slice(h * P, (h + 1) * P)
                nc.vector.tensor_scalar_mul(att[:, hs], S[:, hs], rr[:, h:h + 1])
            # att is 16x16 block-diagonal, so the DVE 32x32 block transpose
            # equals the full per-head transpose.
            attT = p_attT.tile([P, H * P], BF16, tag="attT", name="attT")
            nc.vector.transpose(attT[:], att[:])
            # fill the PE idle while this tile's softmax chain runs on
            # ScalarE/VectorE: MLP matmuls of the previous group + input
            # prep (LN1 + transposes) of the next group.
            drain_c(5)
            if g + 1 < ngroups:
                stage_a_tile(x_t_next, hTg_next, i)
            # o^T pieces: lhsT = v slice [128,(b,s)] x [64 d] -> out [64, 128]
            ps_o = ps_mm.tile([P, C], F32, tag="mm", name="mm")
            for h in range(H):
                nc.tensor.matmul(
                    ps_o[(h % 2) * 64:(h % 2) * 64 + 64,
                         (h // 2) * P:(h // 2) * P + P],
                    v_t[i][:, h * 64:(h + 1) * 64],
                    attT[:, h * P:(h + 1) * P],
                    start=True, stop=True,
                )
            ot = p_oT.tile([P, C], BF16, tag="oT", name="oT")
            oTa.append(ot)
            nc.scalar.activation(ot[:], ps_o[:], AF.Identity)

        # ---- stage B4: proj + residual -> y ----
        for i in range(4):
            ps = ps_mm.tile([P, C], F32, tag="mm", name="mm")
            for p in range(KC):
                nc.tensor.matmul(
                    ps[:], oTa[i][:, p * P:(p + 1) * P], wp[p][:],
                    start=(p == 0), stop=False,
                )
            nc.tensor.matmul(ps[:], ones1[:], bpe[:], start=False, stop=True)
            yt = p_y.tile([P, C], F32, tag="y", name="y")
            y_t.append(yt)
            nc.vector.tensor_tensor(out=yt[:], in0=ps[:], in1=x_t[i][:], op=ALU.add)
        drain_c(8)

        # ---- stage B5: LN2 + transpose ----
        h2Tg = p_h2T.tile([P, KC * GT], FP8, tag="h2T", name="h2T")
        h2Ts = h2Tg[:].rearrange("p (c t) -> p c t", c=KC)
        for i in range(4):
            ht2 = p_h.tile([P, C], BF16, tag="h", name="h")
            layernorm(y_t[i], ht2)
            pst = ps_t.tile([P, KC * P], BF16, tag="pst", name="pst")
            for c in range(KC):
                nc.tensor.transpose(pst[:, c * P:(c + 1) * P],
                                    ht2[:, c * P:(c + 1) * P], ident[:])
            nc.vector.tensor_copy(h2Ts[:, :, i * P:(i + 1) * P], pst[:])
        drain_c(len(pend_c))

        pend_c = make_c_chunks(h2Tg, y_t, g)
        if g + 1 < ngroups:
            x_t, hTg = x_t_next, hTg_next

    drain_c(len(pend_c))


def _fp8_pair(w, ncols):
    """[K, ncols] f32 -> [K//256, 128, 2, ncols] fp8 paired chunks, flattened
    to [K//2, 2*ncols] for a contiguous per-tile DMA."""
    fp8 = ml_dtypes.float8_e4m3
    K = w.shape[0]
    npair = K // 256
    wp = w.reshape(npair, 2, P, ncols).transpose(0, 2, 1, 3)  # [m, p, c, cols]
    wp = np.clip(wp * WS, -240, 240)
    return np.ascontiguousarray(wp.reshape(npair * P, 2 * ncols)).astype(fp8)


def preprocess(inputs):
    """Host-side weight folding. Returns dict of extra device arrays."""
    f32 = np.float32
    bf16 = ml_dtypes.bfloat16
    g1 = np.asarray(inputs["ln1_g"], f32)
    b1n = np.asarray(inputs["ln1_b"], f32)
    g2 = np.asarray(inputs["ln2_g"], f32)
    b2n = np.asarray(inputs["ln2_b"], f32)
    wq = np.asarray(inputs["wq"], f32).transpose(1, 0, 2).reshape(C, HD)
    wk = np.asarray(inputs["wk"], f32).transpose(1, 0, 2).reshape(C, HD)
    wv = np.asarray(inputs["wv"], f32).transpose(1, 0, 2).reshape(C, HD)
    w_proj = np.asarray(inputs["w_proj"], f32)
    b_proj = np.asarray(inputs["b_proj"], f32)
    w1 = np.asarray(inputs["w1"], f32)
    b1 = np.asarray(inputs["b1"], f32)
    w2 = np.asarray(inputs["w2"], f32)
    b2 = np.asarray(inputs["b2"], f32)

    wqg = wq * g1[:, None]
    wkg = wk * g1[:, None]
    wvg = wv * g1[:, None]
    w1g = w1 * g2[:, None]

    wqk = np.concatenate([wqg, wkg], axis=1)          # [C, 1024]
    bqk = b1n @ wqk                                   # [1024]
    bias_v = b1n @ wvg                                # [512]
    bpe = (bias_v @ w_proj + b_proj)[None, :]         # [1, 512]
    bias1 = b2n @ w1g + b1                            # [2048]

    # additive mask: 0 on allowed (block-diag causal), -300 elsewhere
    mask = np.full((P, P), -300.0, f32)
    tril = np.tril(np.ones((T, T), bool))
    for b in range(P // T):
        blk = mask[b * T:(b + 1) * T, b * T:(b + 1) * T]
        blk[tril] = 0.0

    return {
        "wqk": _fp8_pair(wqk, 2 * HD),
        "wv": _fp8_pair(wvg, HD),
        "wp": w_proj.astype(bf16),
        "w1": _fp8_pair(w1g, M1),
        # r is evicted as WS*relu(.); w2/WS and WS*b1 compensate so the relu
        # eviction needs no separate descale op (VectorE tensor_scalar has
        # only two scalar slots).
        "w2": (w2 / WS).astype(bf16),
        "bqk": bqk.astype(f32),
        "b1": (bias1 * WS).astype(f32),
        "b2": b2.astype(f32),
        "bpe": bpe.astype(bf16),
        "mask": np.tile(mask, (1, KC)).astype(bf16),
        "ident": np.eye(P, dtype=bf16),
    }


def _patch_act_tables():
    """Make every activation func we use resolve to the single table set
    `natural_log_exp_and_others` (it contains Ln, Exp, Identity and Relu),
    so bacc's table-load pass emits one load instead of thrashing between
    `natural_log` and `exp_and_others` (~2.7us per switch). Indices of the
    table list are preserved so act_func_set_ids stay valid."""
    import concourse.bacc as _bacc_mod
    import concourse.hw_specs as _hw
    if getattr(_bacc_mod, "_ant_act_tables_patched", False):
        return
    _orig = _hw.get_activation_tables
    ours = {AF.Ln, AF.Exp, AF.Identity, AF.Relu, AF.Copy}

    def patched(arch):
        tables = _orig(arch)
        out = {}
        for name, funcs in tables.items():
            if name == "natural_log_exp_and_others":
                out[name] = funcs
            else:
                out[name] = funcs - ours
        return out

    _bacc_mod.get_activation_tables = patched
    _bacc_mod._ant_act_tables_patched = True


def build(ntok=NTOK_FULL):
    """Build the Bass program; returns nc."""
    from contextlib import ExitStack

    _patch_act_tables()
    nc = bacc.Bacc("TRN2", target_bir_lowering=False, debug=False,
                   enable_asserts=False, num_devices=NCORES)
    ins = {
        "x": nc.dram_tensor("x", [ntok, C], F32, kind="ExternalInput").ap(),
        "wqk": nc.dram_tensor("wqk", [2 * P, 2 * 2 * HD], FP8,
                              kind="ExternalInput").ap(),
        "wv": nc.dram_tensor("wv", [2 * P, 2 * HD], FP8,
                             kind="ExternalInput").ap(),
        "wp": nc.dram_tensor("wp", [HD, C], BF16, kind="ExternalInput").ap(),
        "w1": nc.dram_tensor("w1", [2 * P, 2 * M1], FP8,
                             kind="ExternalInput").ap(),
        "w2": nc.dram_tensor("w2", [M1, C], BF16, kind="ExternalInput").ap(),
        "bqk": nc.dram_tensor("bqk", [2 * HD], F32, kind="ExternalInput").ap(),
        "b1": nc.dram_tensor("b1", [M1], F32, kind="ExternalInput").ap(),
        "b2": nc.dram_tensor("b2", [C], F32, kind="ExternalInput").ap(),
        "bpe": nc.dram_tensor("bpe", [1, C], BF16, kind="ExternalInput").ap(),
        "mask": nc.dram_tensor("mask", [P, KC * P], BF16, kind="ExternalInput").ap(),
        "ident": nc.dram_tensor("ident", [P, P], BF16, kind="ExternalInput").ap(),
    }
    outs = {
        "out": nc.dram_tensor("out", [ntok, C], F32, kind="ExternalOutput").ap(),
    }
    with ExitStack() as ctx:
        tc = ctx.enter_context(tile.TileContext(nc))
        emit_block(ctx, tc, outs, ins, ntok)
    nc.finalize()
    return nc


def kernel(**inputs):
    x = np.ascontiguousarray(np.asarray(inputs["x"], np.float32))
    consts = preprocess(inputs)
    nc = build(NTOK_FULL)
    xs = x.reshape(NCORES, NTOK_FULL, C)
    in_maps = [dict(consts, x=np.ascontiguousarray(xs[c])) for c in range(NCORES)]
    res = run_bass_kernel_spmd(nc, in_maps, core_ids=list(range(NCORES)))
    out = np.stack([res.results[c]["out"] for c in range(NCORES)], axis=0)
    return out.reshape(B, T, C).astype(np.float32)


if __name__ == "__main__":
    rng = np.random.default_rng(0)
    fake = {
        "x": rng.standard_normal((B, T, C), dtype=np.float32),
        "ln1_g": np.ones(C, np.float32), "ln1_b": np.zeros(C, np.float32),
        "wq": rng.standard_normal((H, C, D), dtype=np.float32) * 0.02,
        "wk": rng.standard_normal((H, C, D), dtype=np.float32) * 0.02,
        "wv": rng.standard_normal((H, C, D), dtype=np.float32) * 0.02,
        "w_proj": rng.standard_normal((HD, C), dtype=np.float32) * 0.02,
        "b_proj": np.zeros(C, np.float32),
        "ln2_g": np.ones(C, np.float32), "ln2_b": np.zeros(C, np.float32),
        "w1": rng.standard_normal((C, M1), dtype=np.float32) * 0.02,
        "b1": np.zeros(M1, np.float32),
        "w2": rng.standard_normal((M1, C), dtype=np.float32) * 0.02,
        "b2": np.zeros(C, np.float32),
    }
    out = kernel(**fake)
    print("kernel ran, out shape", out.shape)
